# revision 28
# baseline (speedup 1.0000x reference)
"""CenterLoss update kernel for Trainium2, 8-core SPMD.

Reference computation (N=16384 samples, C=10000 classes, D=128 dims):
    embeded_labels = labels @ center          # [N,D] gather via one-hot
    diff = embeded_labels - embeded_preds
    grad = (labels.T @ diff) / (counts + 1)   # counts = labels.T @ ones
    out  = center - 0.5 * grad

Because each row of ``labels`` is one-hot, ``labels.T @ labels == diag(counts)``,
so the whole thing collapses to a single pass over ``labels``:

    S      = labels.T @ embeded_preds         # [C,D] per-class sum of preds
    counts = column sums of labels            # [C]
    out    = beta * center + gamma * S
             beta  = 1 - 0.5*counts/(counts+1)
             gamma = 0.5/(counts+1)

Sharding: by CLASS, not by batch.  Core i gets the full preds (8MB, a 3%
traffic overhead) plus its own 1250 label *columns* and center rows, and
computes its S shard and counts completely locally -- no collective, no
cross-core reduction, no staging, and no ~30us fixed-cost collective exposed
at the tail (a ReduceScatter variant of this kernel measured 414us against
this design's much shorter critical path).  The host hands each core a
*contiguous* copy of its label column slice, so every k-tile DMA is one
contiguous 640KB read -- the ideal HBM stream pattern.

The 655MB ``labels`` tensor is streamed through the PE exactly once as the
moving matmul operand (computing S.T = preds.T @ labels, accumulating all
128 k-tiles into one PSUM group) in a single fp32r pass (~1e-4 relative
error, far inside the 2e-2 gate).  Per-partition partial counts accumulate
on the vector engine; at the tail they are reduced by a ones matmul (via an
fp16 shadow: 1 PE cycle/row, and class counts here are far below 2048 so
fp16 keeps them exact) and the update is applied per 128-class chunk.
"""

import numpy as np

N, C, D = 16384, 10000, 128
NCORES = 8
CSH = C // NCORES       # 1250 classes per core
LR = 0.5
P = 128
KT = N // P             # 128 k-tiles over the full batch


def _chunks(width, step=512):
    out = []
    c0 = 0
    while c0 < width:
        out.append((c0, min(step, width - c0)))
        c0 += step
    return out


def build_program(n=N, csh=CSH, d=D, ncores=NCORES):
    """Build the SPMD Bass program (identical on every core)."""
    import concourse.bacc as bacc
    import concourse.mybir as mybir
    import concourse.tile as tile
    from concourse.masks import make_identity

    f32 = mybir.dt.float32
    f32r = mybir.dt.float32r
    f16 = mybir.dt.float16
    mult = mybir.AluOpType.mult
    add = mybir.AluOpType.add

    assert n % P == 0

    nc = bacc.Bacc(
        "TRN2",
        target_bir_lowering=False,
        debug=False,
        num_devices=ncores,
    )

    # preds/labels are declared float32r (same bits as the host's fp32) so
    # plain HWDGE DMAs can feed fp32r matmuls at full speed (1 cycle/row vs 4
    # for fp32); skipping the true mantissa rounding costs ~1e-4 relative
    # error, far inside the 2e-2 gate.
    preds = nc.dram_tensor("preds", [n, d], f32r, kind="ExternalInput").ap()
    labels = nc.dram_tensor("labels", [n, csh], f32r, kind="ExternalInput").ap()
    center = nc.dram_tensor("center", [csh, d], f32, kind="ExternalInput").ap()
    out = nc.dram_tensor("out", [csh, d], f32, kind="ExternalOutput").ap()

    with tile.TileContext(nc) as tc:
        with (
            tc.tile_pool(name="const", bufs=1) as const_pool,
            tc.tile_pool(name="lab", bufs=8) as lab_pool,
            tc.tile_pool(name="psum", bufs=1, space="PSUM") as psum,
            tc.tile_pool(name="p3", bufs=2) as p3_pool,
        ):
            identity = const_pool.tile([P, P], f32, name="identity")
            make_identity(nc, identity[:])
            ones_h = const_pool.tile([P, 1], f16, name="ones_h")
            nc.vector.memset(ones_h[:], 1.0)

            # full preds as KT stationary [K=128, M=d] tiles (64KB/partition)
            preds_sb = const_pool.tile([P, KT * d], f32r, name="preds_sb")
            # two partial-count accumulators: the 128-add serial chain at
            # ~1.8us/add saturates the DVE against the 1.9us/k-tile DMA
            # pace, so give every 4th tile to GpSimd (whose adds run ~4us --
            # a 50/50 split makes IT the pacer)
            counts_a = const_pool.tile([P, csh], f32, name="counts_a")
            counts_b = const_pool.tile([P, csh], f32, name="counts_b")

            nt3 = (csh + P - 1) // P
            ctr_tiles = []

            # S.T accumulates across all 128 k-tiles in one PSUM group.
            # Width padded to 3 full PSUM banks: matmul outputs may not cross
            # a bank boundary, and the 226-col remainder chunk would run at
            # 1/4 throughput (fp32r needs >=256 moving columns) -- so run
            # three full 512-wide matmuls instead and let the pad columns
            # compute garbage that is never read.
            cpad = 3 * 512
            st_psum = psum.tile([d, cpad], f32, name="st_psum", tag="st",
                                space="PSUM")

            # ---------------- phase 1: stream labels ----------------
            for t in range(KT):
                # just-in-time preds tile keeps the ring mostly-labels
                nc.sync.dma_start(
                    out=preds_sb[:, t * d:(t + 1) * d],
                    in_=preds[t * P:(t + 1) * P, :],
                )
                lab_t = lab_pool.tile([P, cpad], f32r, name=f"lab_{t}", tag="lab")
                nc.sync.dma_start(
                    out=lab_t[:, 0:csh], in_=labels[t * P:(t + 1) * P, :]
                )
                for c0 in (0, 512, 1024):
                    nc.tensor.matmul(
                        out=st_psum[:, c0:c0 + 512],
                        lhsT=preds_sb[:, t * d:(t + 1) * d],
                        rhs=lab_t[:, c0:c0 + 512],
                        start=(t == 0),
                        stop=(t == KT - 1),
                    )
                on_gps = t % 4 == 3
                eng = nc.gpsimd if on_gps else nc.vector
                acc = counts_b if on_gps else counts_a
                if t == (3 if on_gps else 0):
                    eng.tensor_copy(out=acc[:], in_=lab_t[:, 0:csh].bitcast(f32))
                else:
                    eng.tensor_add(
                        out=acc[:], in0=acc[:], in1=lab_t[:, 0:csh].bitcast(f32)
                    )
                if t == 96:
                    # prefetch the center tiles for the tail update
                    for tt in range(nt3):
                        w = min(P, csh - tt * P)
                        ctr_t = p3_pool.tile(
                            [P, d], f32, name=f"ctr_{tt}", tag="ctr", bufs=nt3
                        )
                        ctr_tiles.append(ctr_t)
                        nc.gpsimd.dma_start(
                            out=ctr_t[0:w, :], in_=center[tt * P:tt * P + w, :]
                        )

            # ---------------- tail: counts + update ----------------
            # fp16 shadows of the counts so the ones-matmul runs at
            # 1 cycle/row instead of fp32's 4 (class counts here are far
            # below 2048, so fp16 keeps them exact); cast on the idle ACT
            # engine so the busy DVE/GpSimd queues aren't in the tail path
            counts_ha = const_pool.tile([P, csh], f16, name="counts_ha")
            nc.scalar.copy(out=counts_ha[:], in_=counts_a[:])
            counts_hb = const_pool.tile([P, csh], f16, name="counts_hb")
            nc.scalar.copy(out=counts_hb[:], in_=counts_b[:])
            cnt_psum = psum.tile([1, csh], f32, name="cnt_psum", tag="cntp",
                                 space="PSUM")
            for c0, w in _chunks(csh):
                nc.tensor.matmul(
                    out=cnt_psum[0:1, c0:c0 + w],
                    lhsT=ones_h[:],
                    rhs=counts_ha[:, c0:c0 + w],
                    start=True,
                    stop=False,
                )
                nc.tensor.matmul(
                    out=cnt_psum[0:1, c0:c0 + w],
                    lhsT=ones_h[:],
                    rhs=counts_hb[:, c0:c0 + w],
                    start=False,
                    stop=True,
                )
            cnt_row = const_pool.tile([1, csh], f32, name="cnt_row")
            nc.scalar.copy(out=cnt_row[:], in_=cnt_psum[:])

            # all nt3 count columns transpose into one [P, nt3] tile so the
            # per-class gamma/beta chain runs as single wide DVE ops
            cntc = psum.tile([P, nt3], f32, name="cntc", tag="cntc",
                             space="PSUM")
            for tt in range(nt3):
                w = min(P, csh - tt * P)
                nc.tensor.transpose(
                    out=cntc[0:w, tt:tt + 1],
                    in_=cnt_row[0:1, tt * P:tt * P + w],
                    identity=identity[0:1, 0:1],
                )
            den = const_pool.tile([P, nt3], f32, name="den")
            nc.vector.tensor_scalar_add(out=den[:], in0=cntc[:], scalar1=1.0)
            rec = const_pool.tile([P, nt3], f32, name="rec")
            nc.vector.reciprocal(out=rec[:], in_=den[:])
            gam = const_pool.tile([P, nt3], f32, name="gam")
            nc.vector.tensor_scalar_mul(out=gam[:], in0=rec[:], scalar1=0.5)
            # beta = 1 - 0.5*cnt/(cnt+1) simplifies to 0.5 + 0.5*rec
            bet = const_pool.tile([P, nt3], f32, name="bet")
            nc.vector.tensor_scalar_add(out=bet[:], in0=gam[:], scalar1=0.5)

            st_sb = const_pool.tile([P, nt3 * P], f32, name="st_sb")
            for tt in range(nt3):
                w = min(P, csh - tt * P)
                # chunked PSUM evacuation so the first transpose starts early
                nc.scalar.copy(
                    out=st_sb[:, tt * P:tt * P + w],
                    in_=st_psum[:, tt * P:tt * P + w],
                )
                ctr_t = ctr_tiles[tt]
                trp = psum.tile([P, d], f32, name=f"trp_{tt}", tag="trp",
                                space="PSUM")
                nc.tensor.transpose(
                    out=trp[0:w, 0:d],
                    in_=st_sb[:, tt * P:tt * P + w],
                    identity=identity[:, 0:d],
                )
                o1 = p3_pool.tile([P, d], f32, name=f"o1_{tt}", tag="o1")
                nc.vector.tensor_scalar_mul(
                    out=o1[0:w, :], in0=ctr_t[0:w, :], scalar1=bet[0:w, tt:tt + 1]
                )
                ou = p3_pool.tile([P, d], f32, name=f"ou_{tt}", tag="ou")
                nc.vector.scalar_tensor_tensor(
                    out=ou[0:w, :], in0=trp[0:w, 0:d], scalar=gam[0:w, tt:tt + 1],
                    in1=o1[0:w, :], op0=mult, op1=add,
                )
                nc.gpsimd.dma_start(
                    out=out[tt * P:tt * P + w, :], in_=ou[0:w, 0:d]
                )

    nc.compile()
    return nc


_PROGRAM = None
LAST_RESULTS = None  # BassKernelResults from the most recent run (for test.py)


def _get_program():
    global _PROGRAM
    if _PROGRAM is None:
        _PROGRAM = build_program()
    return _PROGRAM


def kernel(embeded_preds, labels, center):
    from concourse.bass_utils import run_bass_kernel_spmd

    global LAST_RESULTS
    preds = np.ascontiguousarray(np.asarray(embeded_preds, dtype=np.float32))
    lab = np.ascontiguousarray(np.asarray(labels, dtype=np.float32))
    ctr = np.ascontiguousarray(np.asarray(center, dtype=np.float32))
    assert preds.shape == (N, D) and lab.shape == (N, C) and ctr.shape == (C, D)

    nc = _get_program()
    in_maps = [
        {
            "preds": preds,
            "labels": np.ascontiguousarray(lab[:, i * CSH:(i + 1) * CSH]),
            "center": ctr[i * CSH:(i + 1) * CSH],
        }
        for i in range(NCORES)
    ]
    res = run_bass_kernel_spmd(nc, in_maps, core_ids=list(range(NCORES)))
    LAST_RESULTS = res
    return np.concatenate([res.results[i]["out"] for i in range(NCORES)], axis=0)


# revision 31
# speedup vs baseline: 1.2374x; 1.2374x over previous
"""CenterLoss update kernel for Trainium2, 8-core SPMD.

Reference computation (N=16384 samples, C=10000 classes, D=128 dims):
    embeded_labels = labels @ center          # [N,D] gather via one-hot
    diff = embeded_labels - embeded_preds
    grad = (labels.T @ diff) / (counts + 1)   # counts = labels.T @ ones
    out  = center - 0.5 * grad

Because each row of ``labels`` is one-hot, ``labels.T @ labels == diag(counts)``,
so the whole thing collapses to a single pass over ``labels``:

    S      = labels.T @ embeded_preds         # [C,D] per-class sum of preds
    counts = column sums of labels            # [C]
    out    = beta * center + gamma * S
             beta  = 1 - 0.5*counts/(counts+1)
             gamma = 0.5/(counts+1)

Sharding: by CLASS, not by batch.  Core i gets the full preds (8MB, a 3%
traffic overhead) plus its own 1250 label *columns* and center rows, and
computes its S shard and counts completely locally -- no collective, no
cross-core reduction, no staging, and no ~30us fixed-cost collective exposed
at the tail (a ReduceScatter variant of this kernel measured 414us against
this design's much shorter critical path).  The host hands each core a
*contiguous* copy of its label column slice, so every k-tile DMA is one
contiguous 640KB read -- the ideal HBM stream pattern.

The 655MB ``labels`` tensor is streamed through the PE exactly once as the
moving matmul operand (computing S.T = preds.T @ labels, accumulating all
128 k-tiles into one PSUM group) in a single fp32r pass (~1e-4 relative
error, far inside the 2e-2 gate).  Per-partition partial counts accumulate
on the vector engine; at the tail they are reduced by a ones matmul (via an
fp16 shadow: 1 PE cycle/row, and class counts here are far below 2048 so
fp16 keeps them exact) and the update is applied per 128-class chunk.
"""

import numpy as np

N, C, D = 16384, 10000, 128
NCORES = 8
CSH = C // NCORES       # 1250 classes per core
LR = 0.5
P = 128
KT = N // P             # 128 k-tiles over the full batch


def _chunks(width, step=512):
    out = []
    c0 = 0
    while c0 < width:
        out.append((c0, min(step, width - c0)))
        c0 += step
    return out


def build_program(n=N, csh=CSH, d=D, ncores=NCORES):
    """Build the SPMD Bass program (identical on every core)."""
    import concourse.bacc as bacc
    import concourse.mybir as mybir
    import concourse.tile as tile
    from concourse.masks import make_identity

    f32 = mybir.dt.float32
    f32r = mybir.dt.float32r
    f16 = mybir.dt.float16
    mult = mybir.AluOpType.mult
    add = mybir.AluOpType.add

    assert n % P == 0

    nc = bacc.Bacc(
        "TRN2",
        target_bir_lowering=False,
        debug=False,
        num_devices=ncores,
    )

    # preds/labels are declared float32r (same bits as the host's fp32) so
    # plain HWDGE DMAs can feed fp32r matmuls at full speed (1 cycle/row vs 4
    # for fp32); skipping the true mantissa rounding costs ~1e-4 relative
    # error, far inside the 2e-2 gate.
    preds = nc.dram_tensor("preds", [n, d], f32r, kind="ExternalInput").ap()
    labels = nc.dram_tensor("labels", [n, csh], f32r, kind="ExternalInput").ap()
    center = nc.dram_tensor("center", [csh, d], f32, kind="ExternalInput").ap()
    out = nc.dram_tensor("out", [csh, d], f32, kind="ExternalOutput").ap()

    with tile.TileContext(nc) as tc:
        with (
            tc.tile_pool(name="const", bufs=1) as const_pool,
            tc.tile_pool(name="lab", bufs=8) as lab_pool,
            tc.tile_pool(name="psum", bufs=1, space="PSUM") as psum,
            tc.tile_pool(name="p3", bufs=2) as p3_pool,
        ):
            identity = const_pool.tile([P, P], f32, name="identity")
            make_identity(nc, identity[:])
            ones_h = const_pool.tile([P, 1], f16, name="ones_h")
            nc.vector.memset(ones_h[:], 1.0)

            # full preds as KT stationary [K=128, M=d] tiles (64KB/partition)
            preds_sb = const_pool.tile([P, KT * d], f32r, name="preds_sb")
            # counts accumulate on the DVE only: offloading any share of the
            # adds to GpSimd throttles the whole SBUF (label DMA drops from
            # ~390 to ~305 GB/s and the PE slows too)
            counts_g = const_pool.tile([P, csh], f32, name="counts_g")

            nt3 = (csh + P - 1) // P
            ctr_tiles = []

            # S.T accumulates across all 128 k-tiles in one PSUM group.
            # Width padded to 3 full PSUM banks: matmul outputs may not cross
            # a bank boundary, and the 226-col remainder chunk would run at
            # 1/4 throughput (fp32r needs >=256 moving columns) -- so run
            # three full 512-wide matmuls instead and let the pad columns
            # compute garbage that is never read.
            cpad = 3 * 512
            st_psum = psum.tile([d, cpad], f32, name="st_psum", tag="st",
                                space="PSUM")

            # ---------------- phase 1: stream labels ----------------
            for t in range(KT):
                # just-in-time preds tile keeps the ring mostly-labels
                nc.sync.dma_start(
                    out=preds_sb[:, t * d:(t + 1) * d],
                    in_=preds[t * P:(t + 1) * P, :],
                )
                lab_t = lab_pool.tile([P, cpad], f32r, name=f"lab_{t}", tag="lab")
                nc.sync.dma_start(
                    out=lab_t[:, 0:csh], in_=labels[t * P:(t + 1) * P, :]
                )
                for c0 in (0, 512, 1024):
                    nc.tensor.matmul(
                        out=st_psum[:, c0:c0 + 512],
                        lhsT=preds_sb[:, t * d:(t + 1) * d],
                        rhs=lab_t[:, c0:c0 + 512],
                        start=(t == 0),
                        stop=(t == KT - 1),
                    )
                if t == 0:
                    nc.vector.tensor_copy(
                        out=counts_g[:], in_=lab_t[:, 0:csh].bitcast(f32)
                    )
                else:
                    nc.vector.tensor_add(
                        out=counts_g[:],
                        in0=counts_g[:],
                        in1=lab_t[:, 0:csh].bitcast(f32),
                    )
                if t == 96:
                    # prefetch the center tiles for the tail update
                    for tt in range(nt3):
                        w = min(P, csh - tt * P)
                        ctr_t = p3_pool.tile(
                            [P, d], f32, name=f"ctr_{tt}", tag="ctr", bufs=nt3
                        )
                        ctr_tiles.append(ctr_t)
                        nc.gpsimd.dma_start(
                            out=ctr_t[0:w, :], in_=center[tt * P:tt * P + w, :]
                        )

            # ---------------- tail: counts + update ----------------
            # fp16 shadow of the counts so the ones-matmul runs at
            # 1 cycle/row instead of fp32's 4 (class counts here are far
            # below 2048, so fp16 keeps them exact); cast on the idle ACT
            # engine so the busy DVE queue isn't in the tail path
            counts_h = const_pool.tile([P, csh], f16, name="counts_h")
            nc.scalar.copy(out=counts_h[:], in_=counts_g[:])
            cnt_psum = psum.tile([1, csh], f32, name="cnt_psum", tag="cntp",
                                 space="PSUM")
            for c0, w in _chunks(csh):
                nc.tensor.matmul(
                    out=cnt_psum[0:1, c0:c0 + w],
                    lhsT=ones_h[:],
                    rhs=counts_h[:, c0:c0 + w],
                    start=True,
                    stop=True,
                )
            cnt_row = const_pool.tile([1, csh], f32, name="cnt_row")
            nc.scalar.copy(out=cnt_row[:], in_=cnt_psum[:])

            # all nt3 count columns transpose into one [P, nt3] tile so the
            # per-class gamma/beta chain runs as single wide DVE ops
            cntc = psum.tile([P, nt3], f32, name="cntc", tag="cntc",
                             space="PSUM")
            for tt in range(nt3):
                w = min(P, csh - tt * P)
                nc.tensor.transpose(
                    out=cntc[0:w, tt:tt + 1],
                    in_=cnt_row[0:1, tt * P:tt * P + w],
                    identity=identity[0:1, 0:1],
                )
            den = const_pool.tile([P, nt3], f32, name="den")
            nc.vector.tensor_scalar_add(out=den[:], in0=cntc[:], scalar1=1.0)
            rec = const_pool.tile([P, nt3], f32, name="rec")
            nc.vector.reciprocal(out=rec[:], in_=den[:])
            gam = const_pool.tile([P, nt3], f32, name="gam")
            nc.vector.tensor_scalar_mul(out=gam[:], in0=rec[:], scalar1=0.5)
            # beta = 1 - 0.5*cnt/(cnt+1) simplifies to 0.5 + 0.5*rec
            bet = const_pool.tile([P, nt3], f32, name="bet")
            nc.vector.tensor_scalar_add(out=bet[:], in0=gam[:], scalar1=0.5)

            st_sb = const_pool.tile([P, nt3 * P], f32, name="st_sb")
            for tt in range(nt3):
                w = min(P, csh - tt * P)
                # chunked PSUM evacuation so the first transpose starts early
                nc.scalar.copy(
                    out=st_sb[:, tt * P:tt * P + w],
                    in_=st_psum[:, tt * P:tt * P + w],
                )
                ctr_t = ctr_tiles[tt]
                trp = psum.tile([P, d], f32, name=f"trp_{tt}", tag="trp",
                                space="PSUM")
                nc.tensor.transpose(
                    out=trp[0:w, 0:d],
                    in_=st_sb[:, tt * P:tt * P + w],
                    identity=identity[:, 0:d],
                )
                o1 = p3_pool.tile([P, d], f32, name=f"o1_{tt}", tag="o1")
                nc.vector.tensor_scalar_mul(
                    out=o1[0:w, :], in0=ctr_t[0:w, :], scalar1=bet[0:w, tt:tt + 1]
                )
                ou = p3_pool.tile([P, d], f32, name=f"ou_{tt}", tag="ou")
                nc.vector.scalar_tensor_tensor(
                    out=ou[0:w, :], in0=trp[0:w, 0:d], scalar=gam[0:w, tt:tt + 1],
                    in1=o1[0:w, :], op0=mult, op1=add,
                )
                nc.gpsimd.dma_start(
                    out=out[tt * P:tt * P + w, :], in_=ou[0:w, 0:d]
                )

    nc.compile()
    return nc


_PROGRAM = None
LAST_RESULTS = None  # BassKernelResults from the most recent run (for test.py)


def _get_program():
    global _PROGRAM
    if _PROGRAM is None:
        _PROGRAM = build_program()
    return _PROGRAM


def kernel(embeded_preds, labels, center):
    from concourse.bass_utils import run_bass_kernel_spmd

    global LAST_RESULTS
    preds = np.ascontiguousarray(np.asarray(embeded_preds, dtype=np.float32))
    lab = np.ascontiguousarray(np.asarray(labels, dtype=np.float32))
    ctr = np.ascontiguousarray(np.asarray(center, dtype=np.float32))
    assert preds.shape == (N, D) and lab.shape == (N, C) and ctr.shape == (C, D)

    nc = _get_program()
    in_maps = [
        {
            "preds": preds,
            "labels": np.ascontiguousarray(lab[:, i * CSH:(i + 1) * CSH]),
            "center": ctr[i * CSH:(i + 1) * CSH],
        }
        for i in range(NCORES)
    ]
    res = run_bass_kernel_spmd(nc, in_maps, core_ids=list(range(NCORES)))
    LAST_RESULTS = res
    return np.concatenate([res.results[i]["out"] for i in range(NCORES)], axis=0)


# revision 34
# speedup vs baseline: 1.2643x; 1.0218x over previous
"""CenterLoss update kernel for Trainium2, 8-core SPMD.

Reference computation (N=16384 samples, C=10000 classes, D=128 dims):
    embeded_labels = labels @ center          # [N,D] gather via one-hot
    diff = embeded_labels - embeded_preds
    grad = (labels.T @ diff) / (counts + 1)   # counts = labels.T @ ones
    out  = center - 0.5 * grad

Because each row of ``labels`` is one-hot, ``labels.T @ labels == diag(counts)``,
so the whole thing collapses to a single pass over ``labels``:

    S      = labels.T @ embeded_preds         # [C,D] per-class sum of preds
    counts = column sums of labels            # [C]
    out    = beta * center + gamma * S
             beta  = 1 - 0.5*counts/(counts+1)
             gamma = 0.5/(counts+1)

Sharding: by CLASS, not by batch.  Core i gets the full preds (8MB, a 3%
traffic overhead) plus its own 1250 label *columns* and center rows, and
computes its S shard and counts completely locally -- no collective, no
cross-core reduction, no staging, and no ~30us fixed-cost collective exposed
at the tail (a ReduceScatter variant of this kernel measured 414us against
this design's much shorter critical path).  The host hands each core a
*contiguous* copy of its label column slice, so every k-tile DMA is one
contiguous 640KB read -- the ideal HBM stream pattern.

The 655MB ``labels`` tensor is streamed through the PE exactly once as the
moving matmul operand (computing S.T = preds.T @ labels, accumulating all
128 k-tiles into one PSUM group) in a single fp32r pass (~1e-4 relative
error, far inside the 2e-2 gate).  Per-partition partial counts accumulate
on the vector engine; at the tail they are reduced by a ones matmul (via an
fp16 shadow: 1 PE cycle/row, and class counts here are far below 2048 so
fp16 keeps them exact) and the update is applied per 128-class chunk.
"""

import numpy as np

N, C, D = 16384, 10000, 128
NCORES = 8
CSH = C // NCORES       # 1250 classes per core
LR = 0.5
P = 128
KT = N // P             # 128 k-tiles over the full batch


def _chunks(width, step=512):
    out = []
    c0 = 0
    while c0 < width:
        out.append((c0, min(step, width - c0)))
        c0 += step
    return out


def build_program(n=N, csh=CSH, d=D, ncores=NCORES):
    """Build the SPMD Bass program (identical on every core)."""
    import concourse.bacc as bacc
    import concourse.mybir as mybir
    import concourse.tile as tile
    from concourse.masks import make_identity

    f32 = mybir.dt.float32
    f32r = mybir.dt.float32r
    f16 = mybir.dt.float16
    mult = mybir.AluOpType.mult
    add = mybir.AluOpType.add

    assert n % P == 0

    nc = bacc.Bacc(
        "TRN2",
        target_bir_lowering=False,
        debug=False,
        num_devices=ncores,
    )

    # preds/labels are declared float32r (same bits as the host's fp32) so
    # plain HWDGE DMAs can feed fp32r matmuls at full speed (1 cycle/row vs 4
    # for fp32); skipping the true mantissa rounding costs ~1e-4 relative
    # error, far inside the 2e-2 gate.
    preds = nc.dram_tensor("preds", [n, d], f32r, kind="ExternalInput").ap()
    labels = nc.dram_tensor("labels", [n, csh], f32r, kind="ExternalInput").ap()
    center = nc.dram_tensor("center", [csh, d], f32, kind="ExternalInput").ap()
    out = nc.dram_tensor("out", [csh, d], f32, kind="ExternalOutput").ap()

    with tile.TileContext(nc) as tc:
        with (
            tc.tile_pool(name="const", bufs=1) as const_pool,
            tc.tile_pool(name="lab", bufs=8) as lab_pool,
            tc.tile_pool(name="psum", bufs=1, space="PSUM") as psum,
            tc.tile_pool(name="p3", bufs=2) as p3_pool,
        ):
            identity = const_pool.tile([P, P], f32, name="identity")
            make_identity(nc, identity[:])
            ones_h = const_pool.tile([P, 1], f16, name="ones_h")
            nc.vector.memset(ones_h[:], 1.0)

            # full preds as KT stationary [K=128, M=d] tiles (64KB/partition)
            preds_sb = const_pool.tile([P, KT * d], f32r, name="preds_sb")
            # counts accumulate on the DVE only: offloading any share of the
            # adds to GpSimd throttles the whole SBUF (label DMA drops from
            # ~390 to ~305 GB/s and the PE slows too)
            counts_g = const_pool.tile([P, csh], f32, name="counts_g")

            nt3 = (csh + P - 1) // P
            ctr_tiles = []

            # S.T accumulates across all 128 k-tiles in one PSUM group.
            # Width padded to 3 full PSUM banks: matmul outputs may not cross
            # a bank boundary, and the 226-col remainder chunk would run at
            # 1/4 throughput (fp32r needs >=256 moving columns) -- so run
            # three full 512-wide matmuls instead and let the pad columns
            # compute garbage that is never read.
            cpad = 3 * 512
            st_psum = psum.tile([d, cpad], f32, name="st_psum", tag="st",
                                space="PSUM")

            # ---------------- phase 1: stream labels ----------------
            for t in range(KT):
                # just-in-time preds tile keeps the ring mostly-labels
                nc.sync.dma_start(
                    out=preds_sb[:, t * d:(t + 1) * d],
                    in_=preds[t * P:(t + 1) * P, :],
                )
                lab_t = lab_pool.tile([P, cpad], f32r, name=f"lab_{t}", tag="lab")
                nc.sync.dma_start(
                    out=lab_t[:, 0:csh], in_=labels[t * P:(t + 1) * P, :]
                )
                for c0 in (0, 512, 1024):
                    nc.tensor.matmul(
                        out=st_psum[:, c0:c0 + 512],
                        lhsT=preds_sb[:, t * d:(t + 1) * d],
                        rhs=lab_t[:, c0:c0 + 512],
                        start=(t == 0),
                        stop=(t == KT - 1),
                    )
                if t == 0:
                    nc.vector.tensor_copy(
                        out=counts_g[:], in_=lab_t[:, 0:csh].bitcast(f32)
                    )
                else:
                    nc.vector.tensor_add(
                        out=counts_g[:],
                        in0=counts_g[:],
                        in1=lab_t[:, 0:csh].bitcast(f32),
                    )
                if t == 96:
                    # prefetch the center tiles for the tail update
                    for tt in range(nt3):
                        w = min(P, csh - tt * P)
                        ctr_t = p3_pool.tile(
                            [P, d], f32, name=f"ctr_{tt}", tag="ctr", bufs=nt3
                        )
                        ctr_tiles.append(ctr_t)
                        nc.gpsimd.dma_start(
                            out=ctr_t[0:w, :], in_=center[tt * P:tt * P + w, :]
                        )

            # ---------------- tail: counts + update ----------------
            # S.T PSUM evacuation first (ACT, two wide chunks) so the PE
            # transposes can start while the counts finalize on the DVE
            st_sb = const_pool.tile([P, nt3 * P], f32, name="st_sb")
            for c0, w in ((0, 640), (640, csh - 640)):
                nc.scalar.copy(
                    out=st_sb[:, c0:c0 + w], in_=st_psum[:, c0:c0 + w]
                )

            # fp16 shadow of the counts so the ones-matmul runs at
            # 1 cycle/row instead of fp32's 4 (class counts here are far
            # below 2048, so fp16 keeps them exact); the DVE is idle by now
            counts_h = const_pool.tile([P, csh], f16, name="counts_h")
            nc.vector.tensor_copy(out=counts_h[:], in_=counts_g[:])
            cnt_psum = psum.tile([1, csh], f32, name="cnt_psum", tag="cntp",
                                 space="PSUM")
            for c0, w in _chunks(csh):
                nc.tensor.matmul(
                    out=cnt_psum[0:1, c0:c0 + w],
                    lhsT=ones_h[:],
                    rhs=counts_h[:, c0:c0 + w],
                    start=True,
                    stop=True,
                )
            cnt_row = const_pool.tile([1, csh], f32, name="cnt_row")
            nc.scalar.copy(out=cnt_row[:], in_=cnt_psum[:])

            # all nt3 count columns transpose into one [P, nt3] tile so the
            # per-class gamma/beta chain runs as single wide DVE ops
            cntc = psum.tile([P, nt3], f32, name="cntc", tag="cntc",
                             space="PSUM")
            for tt in range(nt3):
                w = min(P, csh - tt * P)
                nc.tensor.transpose(
                    out=cntc[0:w, tt:tt + 1],
                    in_=cnt_row[0:1, tt * P:tt * P + w],
                    identity=identity[0:1, 0:1],
                )
            den = const_pool.tile([P, nt3], f32, name="den")
            nc.vector.tensor_scalar_add(out=den[:], in0=cntc[:], scalar1=1.0)
            rec = const_pool.tile([P, nt3], f32, name="rec")
            nc.vector.reciprocal(out=rec[:], in_=den[:])
            gam = const_pool.tile([P, nt3], f32, name="gam")
            nc.vector.tensor_scalar_mul(out=gam[:], in0=rec[:], scalar1=0.5)
            # beta = 1 - 0.5*cnt/(cnt+1) simplifies to 0.5 + 0.5*rec
            bet = const_pool.tile([P, nt3], f32, name="bet")
            nc.vector.tensor_scalar_add(out=bet[:], in0=gam[:], scalar1=0.5)

            for tt in range(nt3):
                w = min(P, csh - tt * P)
                ctr_t = ctr_tiles[tt]
                trp = psum.tile([P, d], f32, name=f"trp_{tt}", tag="trp",
                                space="PSUM")
                nc.tensor.transpose(
                    out=trp[0:w, 0:d],
                    in_=st_sb[:, tt * P:tt * P + w],
                    identity=identity[:, 0:d],
                )
                o1 = p3_pool.tile([P, d], f32, name=f"o1_{tt}", tag="o1",
                                  bufs=5)
                nc.vector.tensor_scalar_mul(
                    out=o1[0:w, :], in0=ctr_t[0:w, :], scalar1=bet[0:w, tt:tt + 1]
                )
                ou = p3_pool.tile([P, d], f32, name=f"ou_{tt}", tag="ou",
                                  bufs=5)
                nc.vector.scalar_tensor_tensor(
                    out=ou[0:w, :], in0=trp[0:w, 0:d], scalar=gam[0:w, tt:tt + 1],
                    in1=o1[0:w, :], op0=mult, op1=add,
                )
                nc.gpsimd.dma_start(
                    out=out[tt * P:tt * P + w, :], in_=ou[0:w, 0:d]
                )

    nc.compile()
    return nc


_PROGRAM = None
LAST_RESULTS = None  # BassKernelResults from the most recent run (for test.py)


def _get_program():
    global _PROGRAM
    if _PROGRAM is None:
        _PROGRAM = build_program()
    return _PROGRAM


def kernel(embeded_preds, labels, center):
    from concourse.bass_utils import run_bass_kernel_spmd

    global LAST_RESULTS
    preds = np.ascontiguousarray(np.asarray(embeded_preds, dtype=np.float32))
    lab = np.ascontiguousarray(np.asarray(labels, dtype=np.float32))
    ctr = np.ascontiguousarray(np.asarray(center, dtype=np.float32))
    assert preds.shape == (N, D) and lab.shape == (N, C) and ctr.shape == (C, D)

    nc = _get_program()
    in_maps = [
        {
            "preds": preds,
            "labels": np.ascontiguousarray(lab[:, i * CSH:(i + 1) * CSH]),
            "center": ctr[i * CSH:(i + 1) * CSH],
        }
        for i in range(NCORES)
    ]
    res = run_bass_kernel_spmd(nc, in_maps, core_ids=list(range(NCORES)))
    LAST_RESULTS = res
    return np.concatenate([res.results[i]["out"] for i in range(NCORES)], axis=0)
